# revision 12
# baseline (speedup 1.0000x reference)
"""Trainium2 Bass kernel for nn_Net2_54494545051831 (LocallyConnected2d(7x7)
-> bias -> ReLU -> Linear(28392 -> 10)), on 8 NeuronCores.

Distribution: by output location. Each core owns 3 full output rows
(h = 3c .. 3c+2) plus a 6-7 wide piece of rows 24/25 -> 84/85 locations.
Weights / bias / lw are sharded by location; each core computes two
partial [10, B] pieces of the final linear layer, summed on host.

Per-core layout ("tall band"): for the 3 full output rows, x is reordered
host-side into 28-row blocks per input column w': block row 3*rr + ch =
x[:, ch, 3c + rr, w'] (rr = 0..8 input-row offset), row 27 = 1.0 (bias
folds into the weights).  The contraction for all 3 output rows at one w0
spans rows [28*w0, 28*w0 + 196) - a single group of M = 126 (3 rows x 42
channels) covered by exactly TWO 128-row tiles, using either the natural
tile grid or a 64-row-shifted duplicate grid (both DMA'd) depending on
alignment.  The partial row uses the analogous 22-stride single-row band
with 3 location-groups.  Each group is 2 accumulating matmuls of N=512 at
full PE width; ReLU'd activations for all 58 (group, batch-half) tiles
stay in SBUF; the final Linear runs as batched M=10 matmuls, two at a
time on PE column groups (0,0)/(0,32), accumulating two partial sums the
host adds.  All matmuls bf16 with fp32 accumulation.
"""
import numpy as np
import ml_dtypes

import concourse.mybir as mybir
import concourse.tile as tile
from concourse import bacc
from concourse.bass_utils import run_bass_kernel_spmd

BF16 = mybir.dt.bfloat16
F32 = mybir.dt.float32
RELU = mybir.ActivationFunctionType.Relu

B = 1024
IC, OC, NCLS = 3, 42, 10
KH = KW = 7
OH = OW = 26
NCORES = 8
NB = 2               # two N-chunks of 512
NCHUNK = 512

TS = 28              # tall-band rows per w'-block (27 data + 1 ones)
FSPAN = 6 * TS + 27 + 1    # full-row group window incl bias row (196)
PS = 22              # partial-row band rows per block (21 data + 1 ones)
PWIN = 6 * PS + 21   # single-location window (153)
PSPAN = {2: PWIN + PS, 3: PWIN + 2 * PS}

# band_d tile index map: A0..A6 | B0..B5 | pA0..pA2 | pB0..pB1
N_ATILE, N_BTILE_G, N_PATILE, N_PBTILE = 7, 6, 3, 2
N_BTILE = N_ATILE + N_BTILE_G + N_PATILE + N_PBTILE   # 18
TI_A, TI_B, TI_PA, TI_PB = 0, 7, 13, 16


def _grid_full(w0):
    offA = (TS * w0) % 128
    if offA <= 256 - FSPAN:
        return "A", (TS * w0) // 128
    return "B", (TS * w0 - 64) // 128


def _groups():
    """[(kind, params, tiles, cb)] — kind 'F': params=(w0, grid, k0);
    kind 'P': params=(j0, L, grid, k0). A-grid full groups first."""
    full = [(w0, *_grid_full(w0)) for w0 in range(26)]
    ordered = [f for f in full if f[1] == "A"] + \
              [f for f in full if f[1] == "B"]
    out = []
    cb = 0
    for (w0, g, k0) in ordered:
        base = TI_A if g == "A" else TI_B
        out.append(("F", (w0, g, k0), [base + k0, base + k0 + 1], cb))
        cb += 2
    for (j0, L, g, k0) in [(0, 3, "pA", 0), (6, 2, "pA", 1), (3, 3, "pB", 0)]:
        base = TI_PA if g == "pA" else TI_PB
        out.append(("P", (j0, L, g, k0), [base + k0, base + k0 + 1], cb))
        cb += 2
    return out


GROUPS = _groups()
N_GROUP = len(GROUPS)                       # 29
N_CHUNKS = 2 * N_GROUP                      # 58
WT_COLS = N_CHUNKS * 128
N_WARM = 8                                  # PE warm-up dummy matmuls

_cache = {}


def _build_program():
    if "nc" in _cache:
        return _cache["nc"]

    nc = bacc.Bacc("TRN2", target_bir_lowering=False, debug=False,
                   num_devices=NCORES)
    band_d = nc.dram_tensor("band", [N_BTILE, 128, B], BF16,
                            kind="ExternalInput").ap()
    wt_d = nc.dram_tensor("wt", [128, WT_COLS], BF16,
                          kind="ExternalInput").ap()
    lwp_d = nc.dram_tensor("lwp", [128, N_GROUP * NCLS], BF16,
                           kind="ExternalInput").ap()
    part_d = nc.dram_tensor("part", [74, B], F32,
                            kind="ExternalOutput").ap()

    with tile.TileContext(nc) as tc:
        with (
            tc.tile_pool(name="sb", bufs=1) as sb,
            tc.tile_pool(name="stk_pool", bufs=NB * N_GROUP) as stk_pool,
            tc.tile_pool(name="pp_pool", bufs=4, space="PSUM") as pp_pool,
            tc.tile_pool(name="lin_pool", bufs=2, space="PSUM") as lin_pool,
            tc.tile_pool(name="warm_pool", bufs=1, space="PSUM") as warm_pool,
        ):
            band_s = sb.tile([128, N_BTILE * B], BF16)
            wt_s = sb.tile([128, WT_COLS], BF16)
            lwp_s = sb.tile([128, N_GROUP * NCLS], BF16)
            out_s = sb.tile([74, NB * NCHUNK], F32)
            warm_s = sb.tile([128, NCHUNK], BF16)

            # --- PE warm-up: dummy matmuls on a zeroed tile so HAM
            # un-throttles while the input DMAs are still in flight.
            nc.gpsimd.memset(warm_s, 0.0)
            warm_ps = warm_pool.tile([128, NCHUNK], F32, name="warm_ps")
            for i in range(N_WARM):
                nc.tensor.matmul(warm_ps, warm_s[:, 0:128], warm_s,
                                 start=True, stop=True,
                                 skip_group_check=True)

            # --- input DMAs, interleaved in consumption order.
            def dma_band(t, colsplit=False):
                if colsplit:
                    for nbh in range(NB):
                        nc.sync.dma_start(
                            out=band_s[:, t * B + nbh * NCHUNK:
                                       t * B + (nbh + 1) * NCHUNK],
                            in_=band_d[t][:, nbh * NCHUNK:(nbh + 1) * NCHUNK],
                        )
                else:
                    nc.sync.dma_start(
                        out=band_s[:, t * B:(t + 1) * B], in_=band_d[t])

            def dma_wt(c0, c1):
                nc.sync.dma_start(
                    out=wt_s[:, c0 * 128:c1 * 128],
                    in_=wt_d[:, c0 * 128:c1 * 128],
                )

            dma_wt(0, 4)                 # first two groups' weights
            dma_band(TI_A + 0, colsplit=True)
            dma_band(TI_A + 1, colsplit=True)
            nc.sync.dma_start(out=lwp_s, in_=lwp_d)
            issued = set([TI_A, TI_A + 1])
            wt_hi = 4
            for gi, (kind, params, tiles, cb) in enumerate(GROUPS):
                # stay >=4 groups ahead on weights, in >=12-chunk blocks
                want = cb + 8
                if want > wt_hi and (want - wt_hi >= 12
                                     or want >= N_CHUNKS > wt_hi):
                    hi = min(max(want, wt_hi + 12), N_CHUNKS)
                    dma_wt(wt_hi, hi)
                    wt_hi = hi
                for t in tiles:
                    if t not in issued:
                        dma_band(t)
                        issued.add(t)
            if wt_hi < N_CHUNKS:
                dma_wt(wt_hi, N_CHUNKS)

            # --- LC pass: both batch-halves per group, A-grid groups first.
            stks = {}
            for gi, (kind, params, tiles, cb) in enumerate(GROUPS):
                for nb in range(NB):
                    pp = pp_pool.tile([128, NCHUNK], F32, tag="pp")
                    for ci, bt in enumerate(tiles):
                        rhs = band_s[
                            :, bt * B + nb * NCHUNK:
                               bt * B + nb * NCHUNK + NCHUNK]
                        lhsT = wt_s[:, (cb + ci) * 128:(cb + ci + 1) * 128]
                        nc.tensor.matmul(pp, lhsT, rhs,
                                         start=(ci == 0),
                                         stop=(ci == len(tiles) - 1))
                    stk = stk_pool.tile([128, NCHUNK], BF16, tag="stk")
                    if nb == 0:
                        nc.vector.tensor_scalar_max(stk, pp, 0.0)
                    else:
                        nc.scalar.activation(stk, pp, RELU)
                    stks[(gi, nb)] = stk

            # --- batched linear: three M=10 streams on PE col groups
            # (cols 0/32/64; col group 96-127 has a HW bug, unused).
            n_side = [(N_GROUP + 2) // 3, (N_GROUP + 1) // 3, N_GROUP // 3]
            for nb in range(NB):
                lin_ps = lin_pool.tile([74, NCHUNK], F32, tag="lin",
                                       name=f"lin_ps{nb}")
                for gi in range(N_GROUP):
                    side = gi % 3
                    idx = gi // 3
                    nc.tensor.matmul(
                        lin_ps[32 * side:32 * side + NCLS],
                        lwp_s[:, gi * NCLS:(gi + 1) * NCLS],
                        stks[(gi, nb)],
                        start=(idx == 0), stop=(idx == n_side[side] - 1),
                        tile_position=(0, 32 * side), skip_group_check=True,
                    )
                nc.vector.tensor_copy(
                    out_s[:, nb * NCHUNK:(nb + 1) * NCHUNK], lin_ps)
                nc.sync.dma_start(
                    out=part_d[:, nb * NCHUNK:(nb + 1) * NCHUNK],
                    in_=out_s[:, nb * NCHUNK:(nb + 1) * NCHUNK])

    nc.compile()
    _cache["nc"] = nc
    return nc


def _core_rows(c):
    p0 = (52 * c) // 8
    p1 = (52 * (c + 1)) // 8
    return (24 + p0 // 26, p0 % 26, p1 - p0)


def _prep_core(c, x, W, b, lw4):
    ph, pw0, plen = _core_rows(c)

    # tall band for the 3 full rows ------------------------------------
    fband = np.zeros((N_ATILE * 128, B), dtype=ml_dtypes.bfloat16)
    arr = x[:, :, 3 * c:3 * c + 9, :]          # [B, 3, 9, 32]
    blk = arr.transpose(3, 2, 1, 0).reshape(32, 27, B)   # [w', 3rr+ch, B]
    fb = fband[:32 * TS].reshape(32, TS, B)
    fb[:, :27] = blk.astype(ml_dtypes.bfloat16)
    fb[:, 27] = 1.0

    # 22-stride band for the partial row -------------------------------
    pband = np.zeros(((N_PATILE) * 128, B), dtype=ml_dtypes.bfloat16)
    cj = np.arange(21) // 7
    kij = np.arange(21) % 7
    nblocks = min(14, 32 - pw0)
    wslice = np.arange(nblocks) + pw0
    pblk = x[:, cj[:, None], (ph + kij)[:, None], wslice[None, :]]
    pblk = pblk.transpose(1, 2, 0)             # [21, nblocks, B]
    for bw in range(nblocks):
        e = PS * bw + 21
        pband[PS * bw:e] = pblk[:, bw].astype(ml_dtypes.bfloat16)
        pband[e] = 1.0

    band = np.zeros((N_BTILE, 128, B), dtype=ml_dtypes.bfloat16)
    for k in range(N_ATILE):
        band[TI_A + k] = fband[128 * k:128 * (k + 1)]
    for k in range(N_BTILE_G):
        band[TI_B + k] = fband[64 + 128 * k:64 + 128 * (k + 1)]
    for t in range(N_PATILE):
        band[TI_PA + t] = pband[128 * t:128 * (t + 1)]
    for t in range(N_PBTILE):
        band[TI_PB + t] = pband[64 + 128 * t:64 + 128 * (t + 1)]

    # wt ----------------------------------------------------------------
    wt = np.zeros((128, WT_COLS), dtype=ml_dtypes.bfloat16)
    p = np.arange(128)
    for (kind, params, tiles, cb) in GROUPS:
        for ci in range(2):
            vals = np.zeros((128, 128), dtype=np.float32)
            if kind == "F":
                w0, g, k0 = params
                base = (64 if g == "B" else 0) + 128 * (k0 + ci)
                rel = base + p - TS * w0
                kw = rel // TS
                s = rel % TS
                datav = (rel >= 0) & (rel < FSPAN) & (kw < 7) & (s < 27)
                ch = s % 3
                rr = s // 3
                for r in range(3):
                    kh = rr - r
                    v = datav & (kh >= 0) & (kh < 7)
                    vals[v, 42 * r:42 * r + 42] = \
                        W[:, ch[v], 3 * c + r, w0, kh[v] * 7 + kw[v]].T
                    biasv = (rel >= 0) & (rel < FSPAN) & (kw == 0) & (s == 27)
                    if biasv.any():
                        vals[biasv, 42 * r:42 * r + 42] = b[:, 3 * c + r, w0]
            else:
                j0, L, g, k0 = params
                base = (64 if g == "pB" else 0) + 128 * (k0 + ci)
                for li in range(L):
                    jl = j0 + li
                    if jl >= plen:
                        continue
                    h, w = ph, pw0 + jl
                    rel = base + p - PS * jl
                    kj = rel // PS
                    jj = rel % PS
                    valid = (rel >= 0) & (rel < PWIN) & (jj < 21)
                    vj, vk = jj[valid], kj[valid]
                    Wl = W[:, :, h, w, :]
                    vals[valid, 42 * li:42 * li + 42] = \
                        Wl[:, vj // 7, (vj % 7) * 7 + vk].T
                    biasv = (rel >= 0) & (rel < PWIN) & (jj == 21) & (kj == 0)
                    if biasv.any():
                        vals[biasv, 42 * li:42 * li + 42] = b[:, h, w]
            wt[:, (cb + ci) * 128:(cb + ci + 1) * 128] = \
                vals.astype(ml_dtypes.bfloat16)

    # lwp ----------------------------------------------------------------
    lwp = np.zeros((128, N_GROUP * NCLS), dtype=ml_dtypes.bfloat16)
    for gi, (kind, params, tiles, cb) in enumerate(GROUPS):
        if kind == "F":
            w0 = params[0]
            for r in range(3):
                lwp[42 * r:42 * r + 42, gi * NCLS:(gi + 1) * NCLS] = \
                    lw4[:, :, 3 * c + r, w0].T.astype(ml_dtypes.bfloat16)
        else:
            j0, L = params[0], params[1]
            for li in range(L):
                jl = j0 + li
                if jl >= plen:
                    continue
                lwp[42 * li:42 * li + 42, gi * NCLS:(gi + 1) * NCLS] = \
                    lw4[:, :, ph, pw0 + jl].T.astype(ml_dtypes.bfloat16)
    return {"band": band, "wt": wt, "lwp": lwp}


def _run(x, W, b, lw, lb, trace=False):
    nc = _build_program()
    x = np.ascontiguousarray(np.asarray(x, dtype=np.float32))
    W = np.asarray(W, dtype=np.float32)
    b = np.asarray(b, dtype=np.float32)
    lw = np.asarray(lw, dtype=np.float32)
    lb = np.asarray(lb, dtype=np.float32)
    lw4 = lw.reshape(NCLS, OC, OH, OW)
    in_maps = [_prep_core(c, x, W, b, lw4) for c in range(NCORES)]
    res = run_bass_kernel_spmd(
        nc, in_maps, list(range(NCORES)), trace=trace,
    )
    part = np.zeros((NCLS, B), dtype=np.float32)
    for c in range(NCORES):
        pc = res.results[c]["part"]
        part += pc[0:NCLS] + pc[32:32 + NCLS] + pc[64:64 + NCLS]
    out = part.T + lb[None, :]
    return out.astype(np.float32), res


def kernel(**inputs):
    out, _ = _run(inputs["x"], inputs["W"], inputs["b"], inputs["lw"],
                  inputs["lb"])
    return out


# revision 17
# speedup vs baseline: 1.0879x; 1.0879x over previous
"""Trainium2 Bass kernel for nn_Net2_54494545051831 (LocallyConnected2d(7x7)
-> bias -> ReLU -> Linear(28392 -> 10)), on 8 NeuronCores.

Distribution: by output location. Each core owns 3 full output rows
(h = 3c .. 3c+2) plus a 6-7 wide piece of rows 24/25 -> 84/85 locations.
Weights / bias / lw are sharded by location; each core computes two
partial [10, B] pieces of the final linear layer, summed on host.

Per-core layout ("tall band"): for the 3 full output rows, x is reordered
host-side into 28-row blocks per input column w': block row 3*rr + ch =
x[:, ch, 3c + rr, w'] (rr = 0..8 input-row offset), row 27 = 1.0 (bias
folds into the weights).  The contraction for all 3 output rows at one w0
spans rows [28*w0, 28*w0 + 196) - a single group of M = 126 (3 rows x 42
channels) covered by exactly TWO 128-row tiles, using either the natural
tile grid or a 64-row-shifted duplicate grid (both DMA'd) depending on
alignment.  The partial row uses the analogous 22-stride single-row band
with 3 location-groups.  Each group is 2 accumulating matmuls of N=512 at
full PE width; ReLU'd activations for all 58 (group, batch-half) tiles
stay in SBUF; the final Linear runs as batched M=10 matmuls, two at a
time on PE column groups (0,0)/(0,32), accumulating two partial sums the
host adds.  All matmuls bf16 with fp32 accumulation.
"""
import numpy as np
import ml_dtypes

import concourse.mybir as mybir
import concourse.tile as tile
from concourse import bacc
from concourse.bass_utils import run_bass_kernel_spmd

BF16 = mybir.dt.bfloat16
F32 = mybir.dt.float32
RELU = mybir.ActivationFunctionType.Relu

B = 1024
IC, OC, NCLS = 3, 42, 10
KH = KW = 7
OH = OW = 26
NCORES = 8
NB = 2               # two N-chunks of 512
NCHUNK = 512

TS = 28              # tall-band rows per w'-block (27 data + 1 ones)
FSPAN = 6 * TS + 27 + 1    # full-row group window incl bias row (196)
PS = 22              # partial-row band rows per block (21 data + 1 ones)
PWIN = 6 * PS + 21   # single-location window (153)
PSPAN = {2: PWIN + PS, 3: PWIN + 2 * PS}

# band_d tile index map: A0..A6 | B0..B5 | pA0..pA2 | pB0..pB1
N_ATILE, N_BTILE_G, N_PATILE, N_PBTILE = 7, 6, 3, 2
N_BTILE = N_ATILE + N_BTILE_G + N_PATILE + N_PBTILE   # 18
TI_A, TI_B, TI_PA, TI_PB = 0, 7, 13, 16


def _grid_full(w0):
    offA = (TS * w0) % 128
    if offA <= 256 - FSPAN:
        return "A", (TS * w0) // 128
    return "B", (TS * w0 - 64) // 128


def _groups():
    """[(kind, params, tiles, cb)] — kind 'F': params=(w0, grid, k0);
    kind 'P': params=(j0, L, grid, k0). A-grid full groups first."""
    full = [(w0, *_grid_full(w0)) for w0 in range(26)]
    ordered = [f for f in full if f[1] == "A"] + \
              [f for f in full if f[1] == "B"]
    out = []
    cb = 0
    for (w0, g, k0) in ordered:
        base = TI_A if g == "A" else TI_B
        out.append(("F", (w0, g, k0), [base + k0, base + k0 + 1], cb))
        cb += 2
    for (j0, L, g, k0) in [(0, 3, "pA", 0), (6, 2, "pA", 1), (3, 3, "pB", 0)]:
        base = TI_PA if g == "pA" else TI_PB
        out.append(("P", (j0, L, g, k0), [base + k0, base + k0 + 1], cb))
        cb += 2
    return out


GROUPS = _groups()
N_GROUP = len(GROUPS)                       # 29
N_CHUNKS = 2 * N_GROUP                      # 58
WT_COLS = N_CHUNKS * 128
N_WARM = 8                                  # PE warm-up dummy matmuls

_cache = {}


def _build_program():
    if "nc" in _cache:
        return _cache["nc"]

    nc = bacc.Bacc("TRN2", target_bir_lowering=False, debug=False,
                   num_devices=NCORES)
    band_d = nc.dram_tensor("band", [N_BTILE, 128, B], BF16,
                            kind="ExternalInput").ap()
    wt_d = nc.dram_tensor("wt", [128, WT_COLS], BF16,
                          kind="ExternalInput").ap()
    lwp_d = nc.dram_tensor("lwp", [128, N_GROUP * NCLS], BF16,
                           kind="ExternalInput").ap()
    part_d = nc.dram_tensor("part", [74, B], BF16,
                            kind="ExternalOutput").ap()

    with tile.TileContext(nc) as tc:
        with (
            tc.tile_pool(name="sb", bufs=1) as sb,
            tc.tile_pool(name="stk_pool", bufs=NB * N_GROUP) as stk_pool,
            tc.tile_pool(name="pp_pool", bufs=4, space="PSUM") as pp_pool,
            tc.tile_pool(name="lin_pool", bufs=2, space="PSUM") as lin_pool,
            tc.tile_pool(name="warm_pool", bufs=1, space="PSUM") as warm_pool,
        ):
            band_s = sb.tile([128, N_BTILE * B], BF16)
            wt_s = sb.tile([128, WT_COLS], BF16)
            lwp_s = sb.tile([128, N_GROUP * NCLS], BF16)
            out_s = sb.tile([74, NB * NCHUNK], BF16)
            warm_s = sb.tile([128, NCHUNK], BF16)

            # --- PE warm-up: dummy matmuls on a zeroed tile so HAM
            # un-throttles while the input DMAs are still in flight.
            nc.gpsimd.memset(warm_s, 0.0)
            warm_ps = warm_pool.tile([128, NCHUNK], F32, name="warm_ps")
            for i in range(N_WARM):
                nc.tensor.matmul(warm_ps, warm_s[:, 0:128], warm_s,
                                 start=True, stop=True,
                                 skip_group_check=True)

            # --- input DMAs, interleaved in consumption order.  The head
            # (first two tiles) is quarter-split across queues, with the
            # nb=1 halves issued from the Scalar HWDGE so issue
            # serialization on Sync doesn't gate the first matmuls.
            def dma_band(t):
                nc.sync.dma_start(
                    out=band_s[:, t * B:(t + 1) * B], in_=band_d[t])

            def dma_band_part(eng, t, nbh, p0, p1):
                eng.dma_start(
                    out=band_s[p0:p1, t * B + nbh * NCHUNK:
                               t * B + (nbh + 1) * NCHUNK],
                    in_=band_d[t][p0:p1, nbh * NCHUNK:(nbh + 1) * NCHUNK],
                )

            def dma_wt(c0, c1):
                nc.sync.dma_start(
                    out=wt_s[:, c0 * 128:c1 * 128],
                    in_=wt_d[:, c0 * 128:c1 * 128],
                )

            dma_wt(0, 4)                 # first two groups' weights
            for t in (TI_A, TI_A + 1):
                for (p0, p1) in ((0, 64), (64, 128)):
                    dma_band_part(nc.sync, t, 0, p0, p1)
            for t in (TI_A, TI_A + 1):
                dma_band_part(nc.scalar, t, 1, 0, 128)
            nc.scalar.dma_start(out=lwp_s, in_=lwp_d)
            issued = set([TI_A, TI_A + 1])
            wt_hi = 4
            for gi, (kind, params, tiles, cb) in enumerate(GROUPS):
                # stay >=4 groups ahead on weights, in >=12-chunk blocks
                want = cb + 8
                if want > wt_hi and (want - wt_hi >= 12
                                     or want >= N_CHUNKS > wt_hi):
                    hi = min(max(want, wt_hi + 12), N_CHUNKS)
                    dma_wt(wt_hi, hi)
                    wt_hi = hi
                for t in tiles:
                    if t not in issued:
                        dma_band(t)
                        issued.add(t)
            if wt_hi < N_CHUNKS:
                dma_wt(wt_hi, N_CHUNKS)

            # --- LC pass: both batch-halves per group, A-grid groups
            # first; the first two groups run nb=0 before nb=1 so the
            # head's nb=0 quarter-DMAs alone unblock the first matmuls.
            order = [(0, 0), (1, 0), (0, 1), (1, 1)] + \
                    [(g, nb) for g in range(2, N_GROUP) for nb in range(NB)]
            stks = {}
            for (gi, nb) in order:
                    kind, params, tiles, cb = GROUPS[gi]
                    pp = pp_pool.tile([128, NCHUNK], F32, tag="pp")
                    for ci, bt in enumerate(tiles):
                        rhs = band_s[
                            :, bt * B + nb * NCHUNK:
                               bt * B + nb * NCHUNK + NCHUNK]
                        lhsT = wt_s[:, (cb + ci) * 128:(cb + ci + 1) * 128]
                        nc.tensor.matmul(pp, lhsT, rhs,
                                         start=(ci == 0),
                                         stop=(ci == len(tiles) - 1))
                    stk = stk_pool.tile([128, NCHUNK], BF16, tag="stk")
                    if nb == 0:
                        nc.vector.tensor_scalar_max(stk, pp, 0.0)
                    else:
                        nc.scalar.activation(stk, pp, RELU)
                    stks[(gi, nb)] = stk

            # --- batched linear: three M=10 streams on PE col groups
            # (cols 0/32/64; col group 96-127 has a HW bug, unused).
            n_side = [(N_GROUP + 2) // 3, (N_GROUP + 1) // 3, N_GROUP // 3]
            for nb in range(NB):
                lin_ps = lin_pool.tile([74, NCHUNK], F32, tag="lin",
                                       name=f"lin_ps{nb}")
                for gi in range(N_GROUP):
                    side = gi % 3
                    idx = gi // 3
                    nc.tensor.matmul(
                        lin_ps[32 * side:32 * side + NCLS],
                        lwp_s[:, gi * NCLS:(gi + 1) * NCLS],
                        stks[(gi, nb)],
                        start=(idx == 0), stop=(idx == n_side[side] - 1),
                        tile_position=(0, 32 * side), skip_group_check=True,
                    )
                nc.vector.tensor_copy(
                    out_s[:, nb * NCHUNK:(nb + 1) * NCHUNK], lin_ps)
                nc.sync.dma_start(
                    out=part_d[:, nb * NCHUNK:(nb + 1) * NCHUNK],
                    in_=out_s[:, nb * NCHUNK:(nb + 1) * NCHUNK])

    nc.compile()
    _cache["nc"] = nc
    return nc


def _core_rows(c):
    p0 = (52 * c) // 8
    p1 = (52 * (c + 1)) // 8
    return (24 + p0 // 26, p0 % 26, p1 - p0)


def _prep_core(c, x, W, b, lw4):
    ph, pw0, plen = _core_rows(c)

    # tall band for the 3 full rows ------------------------------------
    fband = np.zeros((N_ATILE * 128, B), dtype=ml_dtypes.bfloat16)
    arr = x[:, :, 3 * c:3 * c + 9, :]          # [B, 3, 9, 32]
    blk = arr.transpose(3, 2, 1, 0).reshape(32, 27, B)   # [w', 3rr+ch, B]
    fb = fband[:32 * TS].reshape(32, TS, B)
    fb[:, :27] = blk.astype(ml_dtypes.bfloat16)
    fb[:, 27] = 1.0

    # 22-stride band for the partial row -------------------------------
    pband = np.zeros(((N_PATILE) * 128, B), dtype=ml_dtypes.bfloat16)
    cj = np.arange(21) // 7
    kij = np.arange(21) % 7
    nblocks = min(14, 32 - pw0)
    wslice = np.arange(nblocks) + pw0
    pblk = x[:, cj[:, None], (ph + kij)[:, None], wslice[None, :]]
    pblk = pblk.transpose(1, 2, 0)             # [21, nblocks, B]
    for bw in range(nblocks):
        e = PS * bw + 21
        pband[PS * bw:e] = pblk[:, bw].astype(ml_dtypes.bfloat16)
        pband[e] = 1.0

    band = np.zeros((N_BTILE, 128, B), dtype=ml_dtypes.bfloat16)
    for k in range(N_ATILE):
        band[TI_A + k] = fband[128 * k:128 * (k + 1)]
    for k in range(N_BTILE_G):
        band[TI_B + k] = fband[64 + 128 * k:64 + 128 * (k + 1)]
    for t in range(N_PATILE):
        band[TI_PA + t] = pband[128 * t:128 * (t + 1)]
    for t in range(N_PBTILE):
        band[TI_PB + t] = pband[64 + 128 * t:64 + 128 * (t + 1)]

    # wt ----------------------------------------------------------------
    wt = np.zeros((128, WT_COLS), dtype=ml_dtypes.bfloat16)
    p = np.arange(128)
    for (kind, params, tiles, cb) in GROUPS:
        for ci in range(2):
            vals = np.zeros((128, 128), dtype=np.float32)
            if kind == "F":
                w0, g, k0 = params
                base = (64 if g == "B" else 0) + 128 * (k0 + ci)
                rel = base + p - TS * w0
                kw = rel // TS
                s = rel % TS
                datav = (rel >= 0) & (rel < FSPAN) & (kw < 7) & (s < 27)
                ch = s % 3
                rr = s // 3
                for r in range(3):
                    kh = rr - r
                    v = datav & (kh >= 0) & (kh < 7)
                    vals[v, 42 * r:42 * r + 42] = \
                        W[:, ch[v], 3 * c + r, w0, kh[v] * 7 + kw[v]].T
                    biasv = (rel >= 0) & (rel < FSPAN) & (kw == 0) & (s == 27)
                    if biasv.any():
                        vals[biasv, 42 * r:42 * r + 42] = b[:, 3 * c + r, w0]
            else:
                j0, L, g, k0 = params
                base = (64 if g == "pB" else 0) + 128 * (k0 + ci)
                for li in range(L):
                    jl = j0 + li
                    if jl >= plen:
                        continue
                    h, w = ph, pw0 + jl
                    rel = base + p - PS * jl
                    kj = rel // PS
                    jj = rel % PS
                    valid = (rel >= 0) & (rel < PWIN) & (jj < 21)
                    vj, vk = jj[valid], kj[valid]
                    Wl = W[:, :, h, w, :]
                    vals[valid, 42 * li:42 * li + 42] = \
                        Wl[:, vj // 7, (vj % 7) * 7 + vk].T
                    biasv = (rel >= 0) & (rel < PWIN) & (jj == 21) & (kj == 0)
                    if biasv.any():
                        vals[biasv, 42 * li:42 * li + 42] = b[:, h, w]
            wt[:, (cb + ci) * 128:(cb + ci + 1) * 128] = \
                vals.astype(ml_dtypes.bfloat16)

    # lwp ----------------------------------------------------------------
    lwp = np.zeros((128, N_GROUP * NCLS), dtype=ml_dtypes.bfloat16)
    for gi, (kind, params, tiles, cb) in enumerate(GROUPS):
        if kind == "F":
            w0 = params[0]
            for r in range(3):
                lwp[42 * r:42 * r + 42, gi * NCLS:(gi + 1) * NCLS] = \
                    lw4[:, :, 3 * c + r, w0].T.astype(ml_dtypes.bfloat16)
        else:
            j0, L = params[0], params[1]
            for li in range(L):
                jl = j0 + li
                if jl >= plen:
                    continue
                lwp[42 * li:42 * li + 42, gi * NCLS:(gi + 1) * NCLS] = \
                    lw4[:, :, ph, pw0 + jl].T.astype(ml_dtypes.bfloat16)
    return {"band": band, "wt": wt, "lwp": lwp}


def _run(x, W, b, lw, lb, trace=False):
    nc = _build_program()
    x = np.ascontiguousarray(np.asarray(x, dtype=np.float32))
    W = np.asarray(W, dtype=np.float32)
    b = np.asarray(b, dtype=np.float32)
    lw = np.asarray(lw, dtype=np.float32)
    lb = np.asarray(lb, dtype=np.float32)
    lw4 = lw.reshape(NCLS, OC, OH, OW)
    in_maps = [_prep_core(c, x, W, b, lw4) for c in range(NCORES)]
    res = run_bass_kernel_spmd(
        nc, in_maps, list(range(NCORES)), trace=trace,
    )
    part = np.zeros((NCLS, B), dtype=np.float32)
    for c in range(NCORES):
        pc = np.asarray(res.results[c]["part"]).astype(np.float32)
        part += pc[0:NCLS] + pc[32:32 + NCLS] + pc[64:64 + NCLS]
    out = part.T + lb[None, :]
    return out.astype(np.float32), res


def kernel(**inputs):
    out, _ = _run(inputs["x"], inputs["W"], inputs["b"], inputs["lw"],
                  inputs["lb"])
    return out
